# revision 21
# baseline (speedup 1.0000x reference)
"""Lorentz multi-head attention on 8 Trainium2 NeuronCores (v2).

Sharding: head-parallel phase 1 (core c computes head c for all batches:
QKV Lorentz projections, Lorentz-inner-product scores, softmax-free
exp-attention, Lorentz-midpoint normalize). A per-batch AllToAll
(head-block -> token-block, bf16 payload) overlaps with the next batch's
phase-1 compute, and phase 2 (concat_logradius fusion + output LorentzFC)
for batch b is interleaved after phase 1 of batch b+1, so only the last
batch's exchange + fusion is exposed at the end.

Phase-2 token assignment is interleaved: core c handles tokens
[b*2048 + c*256 : b*2048 + (c+1)*256) for every batch b; the host
reassembles with a transpose.

Tricks vs v1:
- inputs (x, weights) pre-cast to bf16 on the host: halves DMA, removes
  all on-device fp32->bf16 casts of x.
- q and k projections fused into one M=128 matmul (full PE width); Wk is
  negated on the host so the Lorentz score sign flip folds into the exp
  scale (exp(-SCALE * (-score)) with k_space negated, t_k positive).
- v computed in transposed layout [65, N] like q/k, then rotated to
  token-major via 16 PE transposes (replaces 320 tiny matmuls).
- t-rows (sqrt(1/K + |s|^2)) for q, k, v batched into one Ln + one Exp on
  a [3, N] tile; rows scattered to qa/ka/vT partition 64 via SBUF DMAs.
- attention runs per query-half (1024 cols): scores psum [128, 1024],
  ONE exp per (mi, half), AV accumulates m^T in a [128, 1024] psum tile
  whose tail columns are reused for the Lorentz-radius matmuls.
- radius r = t^2 - |s|^2 computed per 128-token chunk with a single
  sign-vector matmul ([-1 x64, +1]), landing directly in token-partition
  layout for one batched Ln + Exp -> rinv.
"""

import os
import sys

sys.path.insert(0, "/opt/trn_rl_repo")

import numpy as np
import ml_dtypes

import concourse.bass as bass
import concourse.mybir as mybir
import concourse.tile as tile
from concourse import bacc, bass_utils
from concourse.masks import make_identity

# Problem constants (hardcoded per task contract)
B, N, D = 4, 2048, 513
H, DHS = 8, 64
NCORES = 8
KCURV = 0.1
INVK = 10.0
SCALE = 1.0 / np.sqrt(DHS)  # 0.125
S_CONST = 2.8479428291320801  # exp(0.5*(digamma(256)-digamma(32)))
DPAD = 640  # 513 padded to 5*128 (col 513 = constant-1 bias lane)
KC = 5  # contraction chunks of 128
BN = B * N  # 8192 tokens
RPC = BN // NCORES  # 1024 rows per core in phase 2 (256 per batch)
TPB = N // NCORES  # 256 tokens per core per batch
HTOK = TPB // 2  # 128 tokens per core per half-batch A2A
HALF = 1024  # query columns per attention half
F32 = mybir.dt.float32
BF16 = mybir.dt.bfloat16
Ln = mybir.ActivationFunctionType.Ln
Exp = mybir.ActivationFunctionType.Exp

_CACHE = {}
BF = ml_dtypes.bfloat16


def _patch_act_tables(nc):
    # Exp and Ln both live in the natural_log_exp_and_others set; the
    # table-load pass picks the first set containing each function, which
    # splits them across two sets and reloads tables on every Ln<->Exp
    # switch (~1.3us each). Restrict the map so the combined set wins.
    from concourse.hw_specs import get_activation_tables

    try:
        tabs = get_activation_tables(nc.m.arch)
    except Exception:
        return
    if "natural_log_exp_and_others" not in tabs:
        return
    for name, fns in tabs.items():
        if name != "natural_log_exp_and_others":
            fns.discard(Exp)
            fns.discard(Ln)


def _build():
    nc = bacc.Bacc(
        "TRN2", target_bir_lowering=False, debug=False, num_devices=NCORES
    )
    _patch_act_tables(nc)

    xT_ap = nc.dram_tensor("xT", [DPAD, BN], BF16, kind="ExternalInput").ap()
    wqkT_ap = nc.dram_tensor("wqkT", [DPAD, 128], BF16, kind="ExternalInput").ap()
    wvT_ap = nc.dram_tensor("wvT", [DPAD, DHS], BF16, kind="ExternalInput").ap()
    woT_ap = nc.dram_tensor("woT", [DPAD, D - 1], BF16, kind="ExternalInput").ap()
    y_ap = nc.dram_tensor("y", [RPC, D], F32, kind="ExternalOutput").ap()

    with tile.TileContext(nc) as tc:
        with (
            tc.tile_pool(name="const", bufs=1) as constp,
            tc.tile_pool(name="w", bufs=1) as wp,
            tc.tile_pool(name="x", bufs=1) as xp,
            tc.tile_pool(name="qk", bufs=1) as qkp,
            tc.tile_pool(name="att", bufs=1) as atp,
            tc.tile_pool(name="sm", bufs=1) as smp,
            tc.tile_pool(name="p2", bufs=1) as d2p,
            tc.tile_pool(name="ps", bufs=2, space="PSUM") as psp,
            tc.tile_pool(name="sc", bufs=2, space="PSUM") as scp,
            tc.tile_pool(name="mt", bufs=1, space="PSUM") as mtp,
            tc.tile_pool(name="dram", bufs=1, space="DRAM") as dramp,
        ):
            identB = constp.tile([128, 128], BF16)
            make_identity(nc, identB[:])
            signv = constp.tile([65, 1], BF16)
            nc.vector.memset(signv[0:64, :], -1.0)
            nc.vector.memset(signv[64:65, :], 1.0)
            # col 0 selects q rows (0-63), col 32 selects k rows (64-127):
            # activation-engine reads must start at partition 0/32/64, so
            # the k t-sum row lands on partition 32
            selqk = constp.tile([128, 33], BF16)
            nc.vector.memset(selqk[:], 0.0)
            nc.vector.memset(selqk[0:64, 0:1], 1.0)
            nc.vector.memset(selqk[64:128, 32:33], 1.0)
            onesv = constp.tile([64, 1], BF16)
            nc.vector.memset(onesv[:], 1.0)
            bias10 = constp.tile([128, 1], F32)
            nc.vector.memset(bias10[:], INVK)
            biasD = constp.tile([128, 1], F32)
            nc.vector.memset(biasD[:], INVK * (1.0 + H * S_CONST * S_CONST))

            # Weights: [DPAD, S] viewed as [128, KC, S] (host-precast bf16)
            wqkb = wp.tile([128, KC, 128], BF16)
            wvb = wp.tile([128, KC, DHS], BF16)
            wob = wp.tile([128, KC, D - 1], BF16)
            nc.sync.dma_start(wqkb[:], wqkT_ap.rearrange("(k p) s -> p k s", p=128))
            nc.sync.dma_start(wvb[:], wvT_ap.rearrange("(k p) s -> p k s", p=128))
            nc.sync.dma_start(wob[:], woT_ap.rearrange("(k p) s -> p k s", p=128))

            sends = []
            recvs = []
            for b in range(B):
                sends.append(dramp.tile([N, DHS + 1], BF16, tag=f"send{b}",
                                        name=f"send{b}"))
                recvs.append([
                    dramp.tile([NCORES, HTOK, DHS + 1], BF16,
                               tag=f"recv{b}_{h}", name=f"recv{b}_{h}")
                    for h in range(2)
                ])

            qkv = {}
            xts = {}

            # x loads are issued well ahead of each batch so they never
            # queue behind AllToAll traffic on the DMA engines
            def xload(b):
                xtb = []
                for ki in range(KC):
                    t = xp.tile([128, N], BF16, tag="x", bufs=15,
                                name=f"x{b}_{ki}")
                    nc.gpsimd.dma_start(
                        t[:],
                        xT_ap[ki * 128:(ki + 1) * 128, b * N:(b + 1) * N],
                    )
                    xtb.append(t)
                xts[b] = xtb

            # ---- projections (q,k fused; v transposed) + t rows ----
            def proj(b):
                xtb = xts.pop(b)

                qa = qkp.tile([65, N], BF16, tag="qa", bufs=3, name=f"qa{b}")
                ka = qkp.tile([65, N], BF16, tag="ka", bufs=3, name=f"ka{b}")
                vT = qkp.tile([65, N], BF16, tag="vT", bufs=3, name=f"vT{b}")
                # row 0 = q sums, row 32 = k sums, row 64 = v sums
                # (partition-aligned for activation reads; rest is junk)
                tsta = smp.tile([65, N], F32, tag="tsta", bufs=2,
                                name=f"tsta{b}")
                for nj in range(N // 512):
                    js = slice(nj * 512, (nj + 1) * 512)
                    psqk = psp.tile([128, 512], F32, tag="ps", name=f"pqk{b}_{nj}")
                    for ki in range(KC):
                        nc.tensor.matmul(
                            psqk[:], wqkb[:, ki, :], xtb[ki][:, js],
                            start=(ki == 0), stop=(ki == KC - 1),
                        )
                    nc.vector.tensor_copy(qa[0:64, js], psqk[0:64, :])
                    nc.vector.tensor_copy(ka[0:64, js], psqk[64:128, :])
                    sqqk = smp.tile([128, 512], BF16, tag="sqqk", bufs=2,
                                    name=f"sqqk{b}_{nj}")
                    nc.vector.tensor_mul(sqqk[0:64, :], qa[0:64, js],
                                         qa[0:64, js])
                    nc.vector.tensor_mul(sqqk[64:128, :], ka[0:64, js],
                                         ka[0:64, js])
                    psv = psp.tile([64, 512], F32, tag="ps", name=f"pv{b}_{nj}")
                    for ki in range(KC):
                        nc.tensor.matmul(
                            psv[:], wvb[:, ki, :], xtb[ki][:, js],
                            start=(ki == 0), stop=(ki == KC - 1),
                        )
                    nc.vector.tensor_copy(vT[0:64, js], psv[:])
                    sqv = smp.tile([64, 512], BF16, tag="sqv", bufs=2,
                                   name=f"sqv{b}_{nj}")
                    nc.vector.tensor_mul(sqv[:], vT[0:64, js], vT[0:64, js])
                    ptr = psp.tile([65, 512], F32, tag="ps", name=f"ptr{b}_{nj}")
                    nc.tensor.matmul(ptr[0:33, :], selqk[:], sqqk[:],
                                     start=True, stop=True)
                    nc.tensor.matmul(ptr[64:65, :], onesv[:], sqv[:],
                                     start=True, stop=True)
                    nc.vector.tensor_copy(tsta[0:33, js], ptr[0:33, :])
                    nc.vector.tensor_copy(tsta[64:65, js], ptr[64:65, :])
                # t = sqrt(INVK + sum sq): one batched Ln, then one Exp per
                # destination row (direct writes; a DMA scatter here would
                # stall behind AllToAll traffic on the DMA engines)
                tlog = smp.tile([65, N], F32, tag="tlog", bufs=2,
                                name=f"tlog{b}")
                nc.scalar.activation(tlog[:], tsta[:], Ln,
                                     bias=bias10[0:65, :])
                nc.scalar.activation(qa[64:65, :], tlog[0:1, :], Exp,
                                     scale=0.5)
                nc.scalar.activation(ka[64:65, :], tlog[32:33, :], Exp,
                                     scale=0.5)
                nc.scalar.activation(vT[64:65, :], tlog[64:65, :], Exp,
                                     scale=0.5)

                # rotate v to token-major [128, 16, 65]
                va = atp.tile([128, N // 128, DHS + 1], BF16, tag="va",
                              bufs=3, name=f"va{b}")
                for j in range(N // 128):
                    pstv = psp.tile([128, 65], BF16, tag="ps",
                                    name=f"pstv{b}_{j}")
                    nc.tensor.transpose(
                        pstv[:], vT[:, j * 128:(j + 1) * 128],
                        identB[0:65, 0:65],
                    )
                    nc.vector.tensor_copy(va[:, j, :], pstv[:])
                qkv[b] = (qa, ka, va)

            # ---- attention + midpoint + per-half AllToAll ----
            def attention(b):
                qa, ka, va = qkv.pop(b)

                # drain: midpoint normalize + send + AllToAll of one half.
                # Called a couple of mi-steps into the NEXT half's loop so
                # its DVE chain (cast/square) hides behind scores matmuls.
                def drain(h2, mts):
                    qoff = h2 * HALF
                    mTb = atp.tile([65, HALF], BF16, tag="mTb", bufs=2,
                                   name=f"mTb{b}_{h2}")
                    nc.vector.tensor_copy(mTb[:], mts[0:65, :])
                    sqb = atp.tile([65, HALF], BF16, tag="sqb", bufs=2,
                                   name=f"sqb{b}_{h2}")
                    nc.vector.tensor_mul(sqb[:], mTb[:], mTb[:])
                    # r = t^2 - |s|^2 via sign-vector matmul, token layout
                    # (own psum tile so the next half's AV can reset mts
                    # as soon as the mTb copy is done)
                    rps = psp.tile([128, HALF // 128], F32, tag="ps",
                                   name=f"rps{b}_{h2}")
                    for j in range(HALF // 128):
                        nc.tensor.matmul(
                            rps[:, j:j + 1],
                            sqb[:, j * 128:(j + 1) * 128],
                            signv[:],
                            start=True, stop=True,
                        )
                    rl = smp.tile([128, HALF // 128], F32, tag="rl", bufs=2,
                                  name=f"rl{b}_{h2}")
                    nc.scalar.activation(rl[:], rps[:], Ln,
                                         scale=KCURV)
                    rinv = smp.tile([128, HALF // 128], F32, tag="rinv",
                                    bufs=2, name=f"rinv{b}_{h2}")
                    nc.scalar.activation(rinv[:], rl[:], Exp, scale=-0.5)
                    for g in range(HALF // 512):
                        ms = smp.tile([128, 4, DHS + 1], BF16, tag="ms",
                                      bufs=3, name=f"ms{b}_{h2}_{g}")
                        for jj in range(4):
                            j = g * 4 + jj
                            pstr = psp.tile([128, 65], BF16, tag="ps",
                                            name=f"pstr{b}_{h2}_{j}")
                            nc.tensor.transpose(
                                pstr[:], mTb[:, j * 128:(j + 1) * 128],
                                identB[0:65, 0:65],
                            )
                            nc.vector.tensor_scalar_mul(
                                ms[:, jj, :], pstr[:], rinv[:, j:j + 1]
                            )
                        dst = sends[b][qoff + g * 512:qoff + (g + 1) * 512, :]
                        nc.sync.dma_start(
                            dst.rearrange("(c p) d -> p c d", p=128), ms[:]
                        )
                    # exchange this half while the other half computes
                    nc.gpsimd.collective_compute(
                        "AllToAll",
                        mybir.AluOpType.bypass,
                        replica_groups=[list(range(NCORES))],
                        ins=[sends[b][qoff:qoff + HALF, :].opt()],
                        outs=[recvs[b][h2].opt()],
                    )

                pending = None  # (h2, mts) awaiting drain
                for h2 in range(N // HALF):
                    qoff = h2 * HALF
                    mts = mtp.tile([128, HALF], F32, tag="mt", bufs=1,
                                   name=f"mts{b}_{h2}")
                    # software-pipelined: scores(mi) then AV(mi-1), so the
                    # PE never sits behind an exp it is waiting on
                    prev = None
                    for mi in range(N // 128):
                        ks = slice(mi * 128, (mi + 1) * 128)
                        pss = scp.tile([128, HALF], F32, tag="sc", bufs=2,
                                       name=f"pss{b}_{h2}_{mi}")
                        for s in range(HALF // 512):
                            nc.tensor.matmul(
                                pss[:, s * 512:(s + 1) * 512],
                                ka[:, ks],
                                qa[:, qoff + s * 512:qoff + (s + 1) * 512],
                                start=True, stop=True,
                            )
                        if mi == 2 and pending is not None:
                            drain(*pending)
                            pending = None
                        pt = atp.tile([128, HALF], BF16, tag="pt", bufs=3,
                                      name=f"pt{b}_{h2}_{mi}")
                        nc.scalar.activation(pt[:], pss[:], Exp, scale=-SCALE)
                        if prev is not None:
                            pmi, ppt = prev
                            for s in range(HALF // 512):
                                nc.tensor.matmul(
                                    mts[0:65, s * 512:(s + 1) * 512],
                                    va[:, pmi, :],
                                    ppt[:, s * 512:(s + 1) * 512],
                                    start=(pmi == 0), stop=False,
                                )
                        prev = (mi, pt)
                    pmi, ppt = prev
                    for s in range(HALF // 512):
                        nc.tensor.matmul(
                            mts[0:65, s * 512:(s + 1) * 512],
                            va[:, pmi, :],
                            ppt[:, s * 512:(s + 1) * 512],
                            start=False, stop=True,
                        )
                    pending = (h2, mts)
                drain(*pending)

            # ---------------- Phase 2 for one batch ----------------
            def phase2(b):
                rvs = []
                tsA = smp.tile([128, 2], F32, tag="tsA", bufs=2,
                               name=f"tsA{b}")
                for h in range(2):
                    rv = d2p.tile([128, NCORES, DHS + 1], BF16, tag="rv",
                                  bufs=4, name=f"rv{b}_{h}")
                    nc.gpsimd.dma_start(
                        rv[:], recvs[b][h][:].rearrange("j p d -> p j d")
                    )
                    rvs.append(rv)
                    tsq = smp.tile([128, NCORES], F32, tag="tsq", bufs=2,
                                   name=f"tsq{b}_{h}")
                    nc.vector.tensor_mul(tsq[:], rv[:, :, 64], rv[:, :, 64])
                    nc.vector.reduce_sum(tsA[:, h:h + 1], tsq[:],
                                         axis=mybir.AxisListType.X)
                # t' = sqrt(s^2 * sum_h t_h^2 + INVK*(1 + H*s^2))
                lnt = smp.tile([128, 2], F32, tag="lnt", bufs=2,
                               name=f"lnt{b}")
                nc.scalar.activation(
                    lnt[:], tsA[:], Ln, scale=S_CONST * S_CONST, bias=biasD[:]
                )
                tpA = smp.tile([128, 2], F32, tag="tpA", bufs=2,
                               name=f"tpA{b}")
                nc.scalar.activation(tpA[:], lnt[:], Exp, scale=0.5)

                osA = smp.tile([128, 2], F32, tag="osA", bufs=2,
                               name=f"osA{b}")
                for h in range(2):
                    rv = rvs[h]
                    fu = d2p.tile([128, DPAD], BF16, tag="fu", bufs=2,
                                  name=f"fu{b}_{h}")
                    nc.vector.tensor_copy(fu[:, 0:1], tpA[:, h:h + 1])
                    nc.vector.tensor_scalar_mul(
                        fu[:, 1:513].rearrange("p (j s) -> p j s", j=H),
                        rv[:, :, 0:DHS],
                        S_CONST,
                    )
                    nc.vector.memset(fu[:, 513:514], 1.0)
                    nc.vector.memset(fu[:, 514:DPAD], 0.0)

                    ftb = d2p.tile([128, KC, 128], BF16, tag="ftb", bufs=2,
                                   name=f"ftb{b}_{h}")
                    for ki in range(KC):
                        pstf = psp.tile([128, 128], BF16, tag="ps",
                                        name=f"pstf{b}_{h}_{ki}")
                        nc.tensor.transpose(
                            pstf[:], fu[:, ki * 128:(ki + 1) * 128], identB[:]
                        )
                        nc.vector.tensor_copy(ftb[:, ki, :], pstf[:])

                    pso = psp.tile([128, 512], F32, tag="ps",
                                   name=f"pso{b}_{h}")
                    for ki in range(KC):
                        nc.tensor.matmul(
                            pso[:], ftb[:, ki, :], wob[:, ki, :],
                            start=(ki == 0), stop=(ki == KC - 1),
                        )
                    outt = d2p.tile([128, D], F32, tag="outt", bufs=4,
                                    name=f"outt{b}_{h}")
                    nc.vector.tensor_copy(outt[:, 1:D], pso[:])
                    osq = smp.tile([128, 512], BF16, tag="osq", bufs=2,
                                   name=f"osq{b}_{h}")
                    nc.vector.tensor_mul(osq[:], outt[:, 1:D], outt[:, 1:D])
                    nc.vector.reduce_sum(osA[:, h:h + 1], osq[:],
                                         axis=mybir.AxisListType.X)
                    lno = smp.tile([128, 1], F32, tag="lno", bufs=2,
                                   name=f"lno{b}_{h}")
                    nc.scalar.activation(lno[:], osA[:, h:h + 1], Ln,
                                         bias=bias10[:])
                    nc.scalar.activation(outt[:, 0:1], lno[:], Exp, scale=0.5)
                    nc.gpsimd.dma_start(
                        y_ap[b * TPB + h * 128:b * TPB + (h + 1) * 128,
                             0:256],
                        outt[:, 0:256],
                    )
                    nc.scalar.dma_start(
                        y_ap[b * TPB + h * 128:b * TPB + (h + 1) * 128,
                             256:D],
                        outt[:, 256:D],
                    )

            # ------- schedule: proj lookahead + pipelined A2A/phase2 -------
            xload(0)
            xload(1)
            proj(0)
            xload(2)
            proj(1)
            for b in range(B):
                if b + 3 < B:
                    xload(b + 3)
                if b + 2 < B:
                    proj(b + 2)
                if b == B - 1:
                    # keep the last batch's A2A window clear of phase-2 DMAs
                    phase2(b - 1)
                attention(b)
                if 1 <= b < B - 1:
                    phase2(b - 1)
            phase2(B - 1)

    nc.compile()
    return nc


def _prep_inputs(x, Wq, bq, Wk, bk, Wv, bv, Wo, bo):
    xT = np.zeros((DPAD, BN), dtype=np.float32)
    xT[:D, :] = np.ascontiguousarray(x.reshape(BN, D).T)
    xT[D, :] = 1.0
    xTb = xT.astype(BF)

    woT = np.zeros((DPAD, D - 1), dtype=np.float32)
    woT[:D + 1, :] = np.concatenate([Wo.T, bo[None, :]], axis=0)
    woTb = woT.astype(BF)

    in_maps = []
    for h in range(NCORES):
        wqk = np.zeros((DPAD, 128), dtype=np.float32)
        wqk[:D + 1, 0:64] = np.concatenate([Wq[h].T, bq[h][None, :]], axis=0)
        # negated k: folds the Lorentz score sign into the exp scale
        wqk[:D + 1, 64:128] = -np.concatenate([Wk[h].T, bk[h][None, :]],
                                              axis=0)
        wv = np.zeros((DPAD, DHS), dtype=np.float32)
        wv[:D + 1, :] = np.concatenate([Wv[h].T, bv[h][None, :]], axis=0)
        in_maps.append({
            "xT": xTb,
            "wqkT": wqk.astype(BF),
            "wvT": wv.astype(BF),
            "woT": woTb,
        })
    return in_maps


def _run(inputs, trace=False, **kw):
    if "nc" not in _CACHE:
        _CACHE["nc"] = _build()
    nc = _CACHE["nc"]
    in_maps = _prep_inputs(**{k: np.asarray(v) for k, v in inputs.items()})
    res = bass_utils.run_bass_kernel_spmd(
        nc, in_maps, core_ids=list(range(NCORES)), trace=trace, **kw
    )
    y = np.stack([res.results[c]["y"] for c in range(NCORES)], axis=0)
    # y[c, b*256 + h*128 + i, :] holds token b*2048 + h*1024 + c*128 + i
    y = y.reshape(NCORES, B, 2, HTOK, D).transpose(1, 2, 0, 3, 4)
    return np.ascontiguousarray(y.reshape(B, N, D)), res


def kernel(**inputs):
    y, _ = _run(inputs)
    return y


# revision 22
# speedup vs baseline: 1.0442x; 1.0442x over previous
"""Lorentz multi-head attention on 8 Trainium2 NeuronCores (v2).

Sharding: head-parallel phase 1 (core c computes head c for all batches:
QKV Lorentz projections, Lorentz-inner-product scores, softmax-free
exp-attention, Lorentz-midpoint normalize). A per-batch AllToAll
(head-block -> token-block, bf16 payload) overlaps with the next batch's
phase-1 compute, and phase 2 (concat_logradius fusion + output LorentzFC)
for batch b is interleaved after phase 1 of batch b+1, so only the last
batch's exchange + fusion is exposed at the end.

Phase-2 token assignment is interleaved: core c handles tokens
[b*2048 + c*256 : b*2048 + (c+1)*256) for every batch b; the host
reassembles with a transpose.

Tricks vs v1:
- inputs (x, weights) pre-cast to bf16 on the host: halves DMA, removes
  all on-device fp32->bf16 casts of x.
- q and k projections fused into one M=128 matmul (full PE width); Wk is
  negated on the host so the Lorentz score sign flip folds into the exp
  scale (exp(-SCALE * (-score)) with k_space negated, t_k positive).
- v computed in transposed layout [65, N] like q/k, then rotated to
  token-major via 16 PE transposes (replaces 320 tiny matmuls).
- t-rows (sqrt(1/K + |s|^2)) for q, k, v batched into one Ln + one Exp on
  a [3, N] tile; rows scattered to qa/ka/vT partition 64 via SBUF DMAs.
- attention runs per query-half (1024 cols): scores psum [128, 1024],
  ONE exp per (mi, half), AV accumulates m^T in a [128, 1024] psum tile
  whose tail columns are reused for the Lorentz-radius matmuls.
- radius r = t^2 - |s|^2 computed per 128-token chunk with a single
  sign-vector matmul ([-1 x64, +1]), landing directly in token-partition
  layout for one batched Ln + Exp -> rinv.
"""

import os
import sys

sys.path.insert(0, "/opt/trn_rl_repo")

import numpy as np
import ml_dtypes

import concourse.bass as bass
import concourse.mybir as mybir
import concourse.tile as tile
from concourse import bacc, bass_utils
from concourse.masks import make_identity

# Problem constants (hardcoded per task contract)
B, N, D = 4, 2048, 513
H, DHS = 8, 64
NCORES = 8
KCURV = 0.1
INVK = 10.0
SCALE = 1.0 / np.sqrt(DHS)  # 0.125
S_CONST = 2.8479428291320801  # exp(0.5*(digamma(256)-digamma(32)))
DPAD = 640  # 513 padded to 5*128 (col 513 = constant-1 bias lane)
KC = 5  # contraction chunks of 128
BN = B * N  # 8192 tokens
RPC = BN // NCORES  # 1024 rows per core in phase 2 (256 per batch)
TPB = N // NCORES  # 256 tokens per core per batch
HTOK = TPB // 2  # 128 tokens per core per half-batch A2A
HALF = 1024  # query columns per attention half
F32 = mybir.dt.float32
BF16 = mybir.dt.bfloat16
Ln = mybir.ActivationFunctionType.Ln
Exp = mybir.ActivationFunctionType.Exp

_CACHE = {}
BF = ml_dtypes.bfloat16


def _patch_act_tables(nc):
    # Exp and Ln both live in the natural_log_exp_and_others set; the
    # table-load pass picks the first set containing each function, which
    # splits them across two sets and reloads tables on every Ln<->Exp
    # switch (~1.3us each). Restrict the map so the combined set wins.
    from concourse.hw_specs import get_activation_tables

    try:
        tabs = get_activation_tables(nc.m.arch)
    except Exception:
        return
    if "natural_log_exp_and_others" not in tabs:
        return
    for name, fns in tabs.items():
        if name != "natural_log_exp_and_others":
            fns.discard(Exp)
            fns.discard(Ln)


def _build():
    nc = bacc.Bacc(
        "TRN2", target_bir_lowering=False, debug=False, num_devices=NCORES
    )
    _patch_act_tables(nc)

    xT_ap = nc.dram_tensor("xT", [DPAD, BN], BF16, kind="ExternalInput").ap()
    wqkT_ap = nc.dram_tensor("wqkT", [DPAD, 128], BF16, kind="ExternalInput").ap()
    wvT_ap = nc.dram_tensor("wvT", [DPAD, DHS], BF16, kind="ExternalInput").ap()
    woT_ap = nc.dram_tensor("woT", [DPAD, D - 1], BF16, kind="ExternalInput").ap()
    y_ap = nc.dram_tensor("y", [RPC, D], F32, kind="ExternalOutput").ap()

    with tile.TileContext(nc) as tc:
        with (
            tc.tile_pool(name="const", bufs=1) as constp,
            tc.tile_pool(name="w", bufs=1) as wp,
            tc.tile_pool(name="x", bufs=1) as xp,
            tc.tile_pool(name="qk", bufs=1) as qkp,
            tc.tile_pool(name="att", bufs=1) as atp,
            tc.tile_pool(name="sm", bufs=1) as smp,
            tc.tile_pool(name="p2", bufs=1) as d2p,
            tc.tile_pool(name="ps", bufs=2, space="PSUM") as psp,
            tc.tile_pool(name="sc", bufs=2, space="PSUM") as scp,
            tc.tile_pool(name="mt", bufs=1, space="PSUM") as mtp,
            tc.tile_pool(name="dram", bufs=1, space="DRAM") as dramp,
        ):
            identB = constp.tile([128, 128], BF16)
            make_identity(nc, identB[:])
            signv = constp.tile([65, 1], BF16)
            nc.vector.memset(signv[0:64, :], -1.0)
            nc.vector.memset(signv[64:65, :], 1.0)
            # col 0 selects q rows (0-63), col 32 selects k rows (64-127):
            # activation-engine reads must start at partition 0/32/64, so
            # the k t-sum row lands on partition 32
            selqk = constp.tile([128, 33], BF16)
            nc.vector.memset(selqk[:], 0.0)
            nc.vector.memset(selqk[0:64, 0:1], 1.0)
            nc.vector.memset(selqk[64:128, 32:33], 1.0)
            onesv = constp.tile([64, 1], BF16)
            nc.vector.memset(onesv[:], 1.0)
            bias10 = constp.tile([128, 1], F32)
            nc.vector.memset(bias10[:], INVK)
            biasD = constp.tile([128, 1], F32)
            nc.vector.memset(biasD[:], INVK * (1.0 + H * S_CONST * S_CONST))

            # Weights: [DPAD, S] viewed as [128, KC, S] (host-precast bf16)
            wqkb = wp.tile([128, KC, 128], BF16)
            wvb = wp.tile([128, KC, DHS], BF16)
            wob = wp.tile([128, KC, D - 1], BF16)
            nc.sync.dma_start(wqkb[:], wqkT_ap.rearrange("(k p) s -> p k s", p=128))
            nc.sync.dma_start(wvb[:], wvT_ap.rearrange("(k p) s -> p k s", p=128))
            nc.sync.dma_start(wob[:], woT_ap.rearrange("(k p) s -> p k s", p=128))

            sends = []
            recvs = []
            for b in range(B):
                sends.append(dramp.tile([N, DHS + 1], BF16, tag=f"send{b}",
                                        name=f"send{b}"))
                recvs.append([
                    dramp.tile([NCORES, HTOK, DHS + 1], BF16,
                               tag=f"recv{b}_{h}", name=f"recv{b}_{h}")
                    for h in range(2)
                ])

            qkv = {}
            xts = {}

            # x loads are issued well ahead of each batch so they never
            # queue behind AllToAll traffic on the DMA engines
            def xload(b):
                xtb = []
                for ki in range(KC):
                    t = xp.tile([128, N], BF16, tag="x", bufs=15,
                                name=f"x{b}_{ki}")
                    nc.sync.dma_start(
                        t[:],
                        xT_ap[ki * 128:(ki + 1) * 128, b * N:(b + 1) * N],
                    )
                    xtb.append(t)
                xts[b] = xtb

            # ---- projections (q,k fused; v transposed) + t rows ----
            def proj(b):
                xtb = xts.pop(b)

                qa = qkp.tile([65, N], BF16, tag="qa", bufs=3, name=f"qa{b}")
                ka = qkp.tile([65, N], BF16, tag="ka", bufs=3, name=f"ka{b}")
                vT = qkp.tile([65, N], BF16, tag="vT", bufs=3, name=f"vT{b}")
                # row 0 = q sums, row 32 = k sums, row 64 = v sums
                # (partition-aligned for activation reads; rest is junk)
                tsta = smp.tile([65, N], F32, tag="tsta", bufs=2,
                                name=f"tsta{b}")
                for nj in range(N // 512):
                    js = slice(nj * 512, (nj + 1) * 512)
                    psqk = psp.tile([128, 512], F32, tag="ps", name=f"pqk{b}_{nj}")
                    for ki in range(KC):
                        nc.tensor.matmul(
                            psqk[:], wqkb[:, ki, :], xtb[ki][:, js],
                            start=(ki == 0), stop=(ki == KC - 1),
                        )
                    nc.vector.tensor_copy(qa[0:64, js], psqk[0:64, :])
                    nc.vector.tensor_copy(ka[0:64, js], psqk[64:128, :])
                    sqqk = smp.tile([128, 512], BF16, tag="sqqk", bufs=2,
                                    name=f"sqqk{b}_{nj}")
                    nc.vector.tensor_mul(sqqk[0:64, :], qa[0:64, js],
                                         qa[0:64, js])
                    nc.vector.tensor_mul(sqqk[64:128, :], ka[0:64, js],
                                         ka[0:64, js])
                    psv = psp.tile([64, 512], F32, tag="ps", name=f"pv{b}_{nj}")
                    for ki in range(KC):
                        nc.tensor.matmul(
                            psv[:], wvb[:, ki, :], xtb[ki][:, js],
                            start=(ki == 0), stop=(ki == KC - 1),
                        )
                    nc.vector.tensor_copy(vT[0:64, js], psv[:])
                    sqv = smp.tile([64, 512], BF16, tag="sqv", bufs=2,
                                   name=f"sqv{b}_{nj}")
                    nc.vector.tensor_mul(sqv[:], vT[0:64, js], vT[0:64, js])
                    ptr = psp.tile([65, 512], F32, tag="ps", name=f"ptr{b}_{nj}")
                    nc.tensor.matmul(ptr[0:33, :], selqk[:], sqqk[:],
                                     start=True, stop=True)
                    nc.tensor.matmul(ptr[64:65, :], onesv[:], sqv[:],
                                     start=True, stop=True)
                    nc.vector.tensor_copy(tsta[0:33, js], ptr[0:33, :])
                    nc.vector.tensor_copy(tsta[64:65, js], ptr[64:65, :])
                # t = sqrt(INVK + sum sq): one batched Ln, then one Exp per
                # destination row (direct writes; a DMA scatter here would
                # stall behind AllToAll traffic on the DMA engines)
                tlog = smp.tile([65, N], F32, tag="tlog", bufs=2,
                                name=f"tlog{b}")
                nc.scalar.activation(tlog[:], tsta[:], Ln,
                                     bias=bias10[0:65, :])
                nc.scalar.activation(qa[64:65, :], tlog[0:1, :], Exp,
                                     scale=0.5)
                nc.scalar.activation(ka[64:65, :], tlog[32:33, :], Exp,
                                     scale=0.5)
                nc.scalar.activation(vT[64:65, :], tlog[64:65, :], Exp,
                                     scale=0.5)

                # rotate v to token-major [128, 16, 65]
                va = atp.tile([128, N // 128, DHS + 1], BF16, tag="va",
                              bufs=3, name=f"va{b}")
                for j in range(N // 128):
                    pstv = psp.tile([128, 65], BF16, tag="ps",
                                    name=f"pstv{b}_{j}")
                    nc.tensor.transpose(
                        pstv[:], vT[:, j * 128:(j + 1) * 128],
                        identB[0:65, 0:65],
                    )
                    nc.vector.tensor_copy(va[:, j, :], pstv[:])
                qkv[b] = (qa, ka, va)

            # ---- attention + midpoint + per-half AllToAll ----
            def attention(b):
                qa, ka, va = qkv.pop(b)

                # drain: midpoint normalize + send + AllToAll of one half.
                # Called a couple of mi-steps into the NEXT half's loop so
                # its DVE chain (cast/square) hides behind scores matmuls.
                def drain(h2, mts):
                    qoff = h2 * HALF
                    mTb = atp.tile([65, HALF], BF16, tag="mTb", bufs=2,
                                   name=f"mTb{b}_{h2}")
                    nc.vector.tensor_copy(mTb[:], mts[0:65, :])
                    sqb = atp.tile([65, HALF], BF16, tag="sqb", bufs=2,
                                   name=f"sqb{b}_{h2}")
                    nc.vector.tensor_mul(sqb[:], mTb[:], mTb[:])
                    # r = t^2 - |s|^2 via sign-vector matmul, token layout
                    # (own psum tile so the next half's AV can reset mts
                    # as soon as the mTb copy is done)
                    rps = psp.tile([128, HALF // 128], F32, tag="ps",
                                   name=f"rps{b}_{h2}")
                    for j in range(HALF // 128):
                        nc.tensor.matmul(
                            rps[:, j:j + 1],
                            sqb[:, j * 128:(j + 1) * 128],
                            signv[:],
                            start=True, stop=True,
                        )
                    rl = smp.tile([128, HALF // 128], F32, tag="rl", bufs=2,
                                  name=f"rl{b}_{h2}")
                    nc.scalar.activation(rl[:], rps[:], Ln,
                                         scale=KCURV)
                    rinv = smp.tile([128, HALF // 128], F32, tag="rinv",
                                    bufs=2, name=f"rinv{b}_{h2}")
                    nc.scalar.activation(rinv[:], rl[:], Exp, scale=-0.5)
                    for g in range(HALF // 512):
                        ms = smp.tile([128, 4, DHS + 1], BF16, tag="ms",
                                      bufs=3, name=f"ms{b}_{h2}_{g}")
                        for jj in range(4):
                            j = g * 4 + jj
                            pstr = psp.tile([128, 65], BF16, tag="ps",
                                            name=f"pstr{b}_{h2}_{j}")
                            nc.tensor.transpose(
                                pstr[:], mTb[:, j * 128:(j + 1) * 128],
                                identB[0:65, 0:65],
                            )
                            nc.vector.tensor_scalar_mul(
                                ms[:, jj, :], pstr[:], rinv[:, j:j + 1]
                            )
                        dst = sends[b][qoff + g * 512:qoff + (g + 1) * 512, :]
                        nc.sync.dma_start(
                            dst.rearrange("(c p) d -> p c d", p=128), ms[:]
                        )
                    # exchange this half while the other half computes
                    nc.gpsimd.collective_compute(
                        "AllToAll",
                        mybir.AluOpType.bypass,
                        replica_groups=[list(range(NCORES))],
                        ins=[sends[b][qoff:qoff + HALF, :].opt()],
                        outs=[recvs[b][h2].opt()],
                    )

                pending = None  # (h2, mts) awaiting drain
                for h2 in range(N // HALF):
                    qoff = h2 * HALF
                    mts = mtp.tile([128, HALF], F32, tag="mt", bufs=1,
                                   name=f"mts{b}_{h2}")
                    # software-pipelined: scores(mi) then AV(mi-1), so the
                    # PE never sits behind an exp it is waiting on
                    prev = None
                    for mi in range(N // 128):
                        ks = slice(mi * 128, (mi + 1) * 128)
                        pss = scp.tile([128, HALF], F32, tag="sc", bufs=2,
                                       name=f"pss{b}_{h2}_{mi}")
                        for s in range(HALF // 512):
                            nc.tensor.matmul(
                                pss[:, s * 512:(s + 1) * 512],
                                ka[:, ks],
                                qa[:, qoff + s * 512:qoff + (s + 1) * 512],
                                start=True, stop=True,
                            )
                        if mi == 2 and pending is not None:
                            drain(*pending)
                            pending = None
                        pt = atp.tile([128, HALF], BF16, tag="pt", bufs=3,
                                      name=f"pt{b}_{h2}_{mi}")
                        nc.scalar.activation(pt[:], pss[:], Exp, scale=-SCALE)
                        if prev is not None:
                            pmi, ppt = prev
                            for s in range(HALF // 512):
                                nc.tensor.matmul(
                                    mts[0:65, s * 512:(s + 1) * 512],
                                    va[:, pmi, :],
                                    ppt[:, s * 512:(s + 1) * 512],
                                    start=(pmi == 0), stop=False,
                                )
                        prev = (mi, pt)
                    pmi, ppt = prev
                    for s in range(HALF // 512):
                        nc.tensor.matmul(
                            mts[0:65, s * 512:(s + 1) * 512],
                            va[:, pmi, :],
                            ppt[:, s * 512:(s + 1) * 512],
                            start=False, stop=True,
                        )
                    pending = (h2, mts)
                drain(*pending)

            # ---------------- Phase 2 for one batch ----------------
            def phase2(b):
                rvs = []
                tsA = smp.tile([128, 2], F32, tag="tsA", bufs=2,
                               name=f"tsA{b}")
                for h in range(2):
                    rv = d2p.tile([128, NCORES, DHS + 1], BF16, tag="rv",
                                  bufs=4, name=f"rv{b}_{h}")
                    nc.scalar.dma_start(
                        rv[:], recvs[b][h][:].rearrange("j p d -> p j d")
                    )
                    rvs.append(rv)
                    tsq = smp.tile([128, NCORES], F32, tag="tsq", bufs=2,
                                   name=f"tsq{b}_{h}")
                    nc.vector.tensor_mul(tsq[:], rv[:, :, 64], rv[:, :, 64])
                    nc.vector.reduce_sum(tsA[:, h:h + 1], tsq[:],
                                         axis=mybir.AxisListType.X)
                # t' = sqrt(s^2 * sum_h t_h^2 + INVK*(1 + H*s^2))
                lnt = smp.tile([128, 2], F32, tag="lnt", bufs=2,
                               name=f"lnt{b}")
                nc.scalar.activation(
                    lnt[:], tsA[:], Ln, scale=S_CONST * S_CONST, bias=biasD[:]
                )
                tpA = smp.tile([128, 2], F32, tag="tpA", bufs=2,
                               name=f"tpA{b}")
                nc.scalar.activation(tpA[:], lnt[:], Exp, scale=0.5)

                osA = smp.tile([128, 2], F32, tag="osA", bufs=2,
                               name=f"osA{b}")
                for h in range(2):
                    rv = rvs[h]
                    fu = d2p.tile([128, DPAD], BF16, tag="fu", bufs=2,
                                  name=f"fu{b}_{h}")
                    nc.vector.tensor_copy(fu[:, 0:1], tpA[:, h:h + 1])
                    nc.vector.tensor_scalar_mul(
                        fu[:, 1:513].rearrange("p (j s) -> p j s", j=H),
                        rv[:, :, 0:DHS],
                        S_CONST,
                    )
                    nc.vector.memset(fu[:, 513:514], 1.0)
                    nc.vector.memset(fu[:, 514:DPAD], 0.0)

                    ftb = d2p.tile([128, KC, 128], BF16, tag="ftb", bufs=2,
                                   name=f"ftb{b}_{h}")
                    for ki in range(KC):
                        pstf = psp.tile([128, 128], BF16, tag="ps",
                                        name=f"pstf{b}_{h}_{ki}")
                        nc.tensor.transpose(
                            pstf[:], fu[:, ki * 128:(ki + 1) * 128], identB[:]
                        )
                        nc.vector.tensor_copy(ftb[:, ki, :], pstf[:])

                    pso = psp.tile([128, 512], F32, tag="ps",
                                   name=f"pso{b}_{h}")
                    for ki in range(KC):
                        nc.tensor.matmul(
                            pso[:], ftb[:, ki, :], wob[:, ki, :],
                            start=(ki == 0), stop=(ki == KC - 1),
                        )
                    outt = d2p.tile([128, D], F32, tag="outt", bufs=4,
                                    name=f"outt{b}_{h}")
                    nc.vector.tensor_copy(outt[:, 1:D], pso[:])
                    osq = smp.tile([128, 512], BF16, tag="osq", bufs=2,
                                   name=f"osq{b}_{h}")
                    nc.vector.tensor_mul(osq[:], outt[:, 1:D], outt[:, 1:D])
                    nc.vector.reduce_sum(osA[:, h:h + 1], osq[:],
                                         axis=mybir.AxisListType.X)
                    lno = smp.tile([128, 1], F32, tag="lno", bufs=2,
                                   name=f"lno{b}_{h}")
                    nc.scalar.activation(lno[:], osA[:, h:h + 1], Ln,
                                         bias=bias10[:])
                    nc.scalar.activation(outt[:, 0:1], lno[:], Exp, scale=0.5)
                    nc.scalar.dma_start(
                        y_ap[b * TPB + h * 128:b * TPB + (h + 1) * 128,
                             0:256],
                        outt[:, 0:256],
                    )
                    nc.scalar.dma_start(
                        y_ap[b * TPB + h * 128:b * TPB + (h + 1) * 128,
                             256:D],
                        outt[:, 256:D],
                    )

            # ------- schedule: proj lookahead + pipelined A2A/phase2 -------
            xload(0)
            xload(1)
            proj(0)
            xload(2)
            proj(1)
            for b in range(B):
                if b + 3 < B:
                    xload(b + 3)
                if b + 2 < B:
                    proj(b + 2)
                if b == B - 1:
                    # keep the last batch's A2A window clear of phase-2 DMAs
                    phase2(b - 1)
                attention(b)
                if 1 <= b < B - 1:
                    phase2(b - 1)
            phase2(B - 1)

    nc.compile()
    return nc


def _prep_inputs(x, Wq, bq, Wk, bk, Wv, bv, Wo, bo):
    xT = np.zeros((DPAD, BN), dtype=np.float32)
    xT[:D, :] = np.ascontiguousarray(x.reshape(BN, D).T)
    xT[D, :] = 1.0
    xTb = xT.astype(BF)

    woT = np.zeros((DPAD, D - 1), dtype=np.float32)
    woT[:D + 1, :] = np.concatenate([Wo.T, bo[None, :]], axis=0)
    woTb = woT.astype(BF)

    in_maps = []
    for h in range(NCORES):
        wqk = np.zeros((DPAD, 128), dtype=np.float32)
        wqk[:D + 1, 0:64] = np.concatenate([Wq[h].T, bq[h][None, :]], axis=0)
        # negated k: folds the Lorentz score sign into the exp scale
        wqk[:D + 1, 64:128] = -np.concatenate([Wk[h].T, bk[h][None, :]],
                                              axis=0)
        wv = np.zeros((DPAD, DHS), dtype=np.float32)
        wv[:D + 1, :] = np.concatenate([Wv[h].T, bv[h][None, :]], axis=0)
        in_maps.append({
            "xT": xTb,
            "wqkT": wqk.astype(BF),
            "wvT": wv.astype(BF),
            "woT": woTb,
        })
    return in_maps


def _run(inputs, trace=False, **kw):
    if "nc" not in _CACHE:
        _CACHE["nc"] = _build()
    nc = _CACHE["nc"]
    in_maps = _prep_inputs(**{k: np.asarray(v) for k, v in inputs.items()})
    res = bass_utils.run_bass_kernel_spmd(
        nc, in_maps, core_ids=list(range(NCORES)), trace=trace, **kw
    )
    y = np.stack([res.results[c]["y"] for c in range(NCORES)], axis=0)
    # y[c, b*256 + h*128 + i, :] holds token b*2048 + h*1024 + c*128 + i
    y = y.reshape(NCORES, B, 2, HTOK, D).transpose(1, 2, 0, 3, 4)
    return np.ascontiguousarray(y.reshape(B, N, D)), res


def kernel(**inputs):
    y, _ = _run(inputs)
    return y
